# revision 5
# baseline (speedup 1.0000x reference)
"""DQN-GNN kernel for 8 Trainium2 NeuronCores.

Contract: kernel(**inputs) takes the FULL unsharded inputs of
nn_DQN_GNN_53910429499419 and returns the full [2000] float32 output.

Split of work:
  * host (numpy): the L=1 transformer prologue, GCN degree norms, edge
    metadata build, final-MLP epilogue.
  * device (Bass/Tile, SPMD over 8 cores): the 6 GCN layers over
    50000 nodes / ~450K edges at 352 features -- node-sharded, with
    per-layer AllGather of the transformed feature table, dma_gather of
    source rows, and a one-hot*norm matmul segment-sum into PSUM,
    followed by fused LayerNorm+GELU.
"""

import numpy as np
import ml_dtypes

import concourse.bacc as bacc
import concourse.mybir as mybir
import concourse.tile as tile
from concourse.bass_utils import run_bass_kernel_spmd

BF16 = ml_dtypes.bfloat16
F32 = mybir.dt.float32
BF = mybir.dt.bfloat16
I16 = mybir.dt.int16
I32 = mybir.dt.int32

N_NODES = 50000
D = 352
DP = 384              # feature row padded so DP*2 bytes % 256 == 0
IN_DIM = 2000
NCORE = 8
NLAYERS = 6
EPS = 1e-5


# ----------------------------------------------------------------------------
# host math: prologue / epilogue
# ----------------------------------------------------------------------------

def _f32(x):
    return np.asarray(x, np.float32)


def _lin(x, p):
    return x @ _f32(p["w"]).T + _f32(p["b"])


def _mha_l1(x, p):
    # sequence length 1: softmax over a single key == 1, so the attention
    # output is just the value projection passed through the out proj.
    qkv = _lin(x, p["in"])
    v = qkv[:, 2 * D:]
    return _lin(v, p["out"])


def _ln_np(x, p):
    m = x.mean(-1, keepdims=True)
    v = ((x - m) ** 2).mean(-1, keepdims=True)
    return (x - m) / np.sqrt(v + EPS) * _f32(p["g"]) + _f32(p["b"])


def _prologue(state, scores, params):
    x = state[None, :] @ _f32(params["proj"]["w"]).T + _f32(params["proj"]["b"])
    attn1 = _mha_l1(x, params["attn1"])
    t = attn1
    for lp in params["enc"]:
        t = _ln_np(t + _mha_l1(t, lp["attn"]), lp["ln1"])
        ff = np.maximum(_lin(t, lp["ff1"]), 0.0) @ _f32(lp["ff2"]["w"]).T + _f32(lp["ff2"]["b"])
        t = _ln_np(t + ff, lp["ln2"])
    attn2 = _mha_l1(t, params["attn2"])
    h0 = np.concatenate([attn2, scores], axis=0).astype(np.float32)
    return attn1, h0


def _epilogue(pool_sum, attn1, params, n_nodes):
    pooled = pool_sum / n_nodes
    q = np.maximum(_lin(pooled[None], params["mlp1"]), 0.0) \
        @ _f32(params["mlp2"]["w"]).T + _f32(params["mlp2"]["b"])
    return (attn1.mean() * q[0]).astype(np.float32)


def _gcn_edges(edge_index, n_nodes):
    loop = np.arange(n_nodes, dtype=np.int64)
    src = np.concatenate([np.asarray(edge_index[0], np.int64), loop])
    dst = np.concatenate([np.asarray(edge_index[1], np.int64), loop])
    deg = np.bincount(dst, minlength=n_nodes).astype(np.float32)
    dis = 1.0 / np.sqrt(np.maximum(deg, 1e-12))
    norm = (dis[src] * dis[dst]).astype(np.float32)
    return src, dst, norm


# ----------------------------------------------------------------------------
# host metadata: per-core edge tiles for the gather/segment-sum schedule
# ----------------------------------------------------------------------------

def _build_meta(src, dst, norm, n_nodes, ncore):
    """Edges partitioned by dst owner, grouped by (dst_group_of_128, table).

    The per-layer feature table is AllGathered in two pieces: table A holds
    every rank's local rows [0, s_a), table B the rest; each piece has
    < 32768 rows so gather indices fit int16.
    Returns the shared tile-count table T[g][tab] (max over cores, so the
    SPMD program is identical) and the per-core packed index/dst/norm arrays.
    """
    shard = n_nodes // ncore
    s_a = (shard + 1) // 2
    s_b = shard - s_a
    groups = (shard + 127) // 128

    percore = []
    for c in range(ncore):
        lo = c * shard
        m = (dst >= lo) & (dst < lo + shard)
        es, ed, en = src[m], (dst[m] - lo), norm[m]
        r = es // shard
        loc = es - r * shard
        ta = (loc >= s_a).astype(np.int64)
        row = np.where(ta == 0, r * s_a + loc, r * s_b + (loc - s_a))
        assert row.max() < 32768
        g = ed >> 7
        order = np.lexsort((ed, ta, g))
        es, ed, en, ta, row, g = (a[order] for a in (es, ed, en, ta, row, g))
        cnt = np.zeros((groups, 2), np.int64)
        np.add.at(cnt, (g, ta), 1)
        percore.append(dict(ed=ed, en=en, ta=ta, row=row, cnt=cnt))

    cnts = np.stack([p["cnt"] for p in percore])          # [ncore, groups, 2]
    T = np.ceil(cnts.max(axis=0) / 128).astype(np.int64)  # [groups, 2]
    off = np.zeros((groups, 2), np.int64)
    acc = 0
    for g in range(groups):
        for t in (0, 1):
            off[g, t] = acc
            acc += int(T[g, t])
    nslots = acc
    nidx = nslots * 128

    idx_all = np.zeros((ncore, nidx), np.int16)
    dst_all = np.full((ncore, nidx), -1.0, np.float32)
    nrm_all = np.zeros((ncore, nidx), np.float32)
    for c in range(ncore):
        p = percore[c]
        pos = 0
        for g in range(groups):
            for t in (0, 1):
                n = int(p["cnt"][g, t])
                base = int(off[g, t]) * 128
                sl = slice(pos, pos + n)
                idx_all[c, base:base + n] = p["row"][sl].astype(np.int16)
                dst_all[c, base:base + n] = (p["ed"][sl] & 127).astype(np.float32)
                nrm_all[c, base:base + n] = p["en"][sl]
                pos += n
        assert pos == p["ed"].shape[0]

    # idx wrap: entry i of a call at partition i%16, column i//16; replicated
    # over the 8 q7 core groups (partitions 16c..16c+15).
    idx_wrapped = np.zeros((ncore, 128, nidx // 16), np.int16)
    for c in range(ncore):
        w = idx_all[c].reshape(nidx // 16, 16).T
        idx_wrapped[c] = np.tile(w, (8, 1))
    dst_col = np.ascontiguousarray(
        dst_all.reshape(ncore, nslots, 128).transpose(0, 2, 1))
    nrm_col = np.ascontiguousarray(
        nrm_all.reshape(ncore, nslots, 128).transpose(0, 2, 1))
    return T, off, nslots, idx_wrapped, dst_col, nrm_col


# ----------------------------------------------------------------------------
# device program
# ----------------------------------------------------------------------------

def _build_program(n_nodes, ncore, n_layers, T, off, nslots, debug_tab=False):
    shard = n_nodes // ncore
    s_a = (shard + 1) // 2
    s_b = shard - s_a
    groups = (shard + 127) // 128
    nidx = nslots * 128
    rows_a = ncore * s_a
    rows_b = ncore * s_b
    # transform groups are emitted first-half-shard first so AG A can launch
    # while the second half's transforms still run
    ga_split = (s_a + 127) // 128   # first group index whose rows may go to B

    nc = bacc.Bacc("TRN2", target_bir_lowering=False, debug=False,
                   num_devices=ncore)

    h0_d = nc.dram_tensor("h0s", [shard, DP], F32, kind="ExternalInput")
    idx_d = nc.dram_tensor("idx16", [128, nidx // 16], I16, kind="ExternalInput")
    dst_d = nc.dram_tensor("dstloc", [128, nslots], F32, kind="ExternalInput")
    nrm_d = nc.dram_tensor("nrm", [128, nslots], F32, kind="ExternalInput")
    wts_d = nc.dram_tensor("wts", [128, n_layers, 3, D], F32, kind="ExternalInput")
    id_d = nc.dram_tensor("ident", [128, 128], F32, kind="ExternalInput")
    ones_d = nc.dram_tensor("onescol", [128, 1], F32, kind="ExternalInput")
    pool_o = nc.dram_tensor("pool_out", [1, D], F32, kind="ExternalOutput")
    dbg_o = None
    if debug_tab:
        dbg_o = nc.dram_tensor("dbg_tab", [rows_a + rows_b, DP], BF,
                               kind="ExternalOutput")

    with tile.TileContext(nc) as tc:
        with (
            tc.tile_pool(name="const", bufs=1) as constp,
            tc.tile_pool(name="meta", bufs=1) as metap,
            tc.tile_pool(name="gbuf", bufs=3) as gpool,
            tc.tile_pool(name="spool", bufs=4) as spool,
            tc.tile_pool(name="hpool", bufs=2) as hpool,
            tc.tile_pool(name="small", bufs=3) as smallp,
            tc.tile_pool(name="psA", bufs=2, space="PSUM") as psagg,
            tc.tile_pool(name="psT", bufs=2, space="PSUM") as pstp,
            tc.tile_pool(name="psH", bufs=2, space="PSUM") as pshw,
            tc.tile_pool(name="psP", bufs=1, space="PSUM") as pspool,
            tc.tile_pool(name="dram", bufs=1, space="DRAM") as dram,
        ):
            # persistent SBUF state
            idx_t = metap.tile([128, nidx // 16], I16)
            nc.sync.dma_start(idx_t[:], idx_d[:])
            dst_t = metap.tile([128, nslots], F32)
            nc.sync.dma_start(dst_t[:], dst_d[:])
            nrm_t = metap.tile([128, nslots], F32)
            nc.sync.dma_start(nrm_t[:], nrm_d[:])
            wt_t = metap.tile([128, n_layers, 3, D], F32)
            nc.sync.dma_start(wt_t[:], wts_d[:])
            id_t = constp.tile([128, 128], F32)
            nc.sync.dma_start(id_t[:], id_d[:])
            ones_t = constp.tile([128, 1], F32)
            nc.sync.dma_start(ones_t[:], ones_d[:])
            eps_t = constp.tile([128, 1], F32)
            nc.gpsimd.memset(eps_t[:], EPS)
            iota_i = constp.tile([128, 128], I32)
            nc.gpsimd.iota(iota_i[:], pattern=[[1, 128]], channel_multiplier=0)
            iota_f = constp.tile([128, 128], F32)
            nc.vector.tensor_copy(iota_f[:], iota_i[:])

            # DRAM tables (parity-alternated across layers)
            tab_a = [dram.tile([rows_a, DP], BF, addr_space="Shared",
                               tag=f"tab_a{i}", name=f"tab_a{i}")
                     for i in range(n_layers)]
            tab_b = [dram.tile([rows_b, DP], BF, addr_space="Shared",
                               tag=f"tab_b{i}", name=f"tab_b{i}")
                     for i in range(n_layers)]
            own_a = [dram.tile([s_a, DP], BF, tag=f"own_a{i}", name=f"own_a{i}") for i in range(2)]
            own_b = [dram.tile([s_b, DP], BF, tag=f"own_b{i}", name=f"own_b{i}") for i in range(2)]

            ps_pool = pspool.tile([1, D], F32)

            def transform(h_t, g, l, par):
                """h_t: [128, DP] SBUF tile (f32) holding nodes g*128.. ;
                matmul with layer-l weights, write bf16 rows into own_a/b[par]."""
                rows = min(128, shard - g * 128)
                hT = hpool.tile([128, 3, 128], F32, tag="hT")
                for b in range(3):
                    pst = pstp.tile([128, 128], F32, tag="pst")
                    nc.tensor.transpose(pst[:], h_t[:, b * 128:(b + 1) * 128], id_t[:])
                    nc.vector.tensor_copy(hT[:, b, :], pst[:])
                ps_h = pshw.tile([128, D], F32, tag="pshw")
                for b in range(3):
                    nc.tensor.matmul(ps_h[:], hT[:, b, :], wt_t[:, l, b, :],
                                     start=(b == 0), stop=(b == 2))
                st = hpool.tile([128, DP], BF, tag="stage")
                nc.vector.tensor_copy(st[:, :D], ps_h[:])
                r0, r1 = g * 128, g * 128 + rows
                if r1 <= s_a:
                    nc.sync.dma_start(own_a[par][r0:r1, :], st[:rows, :])
                elif r0 >= s_a:
                    nc.sync.dma_start(own_b[par][r0 - s_a:r1 - s_a, :], st[:rows, :])
                else:
                    k = s_a - r0
                    nc.sync.dma_start(own_a[par][r0:s_a, :], st[:k, :])
                    nc.sync.dma_start(own_b[par][0:r1 - s_a, :], st[k:rows, :])

            def allgathers(l):
                nc.gpsimd.collective_compute(
                    "AllGather", mybir.AluOpType.bypass,
                    replica_groups=[list(range(ncore))],
                    ins=[own_a[l % 2][:]], outs=[tab_a[l][:]])
                nc.gpsimd.collective_compute(
                    "AllGather", mybir.AluOpType.bypass,
                    replica_groups=[list(range(ncore))],
                    ins=[own_b[l % 2][:]], outs=[tab_b[l][:]])

            # ---- phase 0: transform input h0 with layer-0 weights ----
            for g in range(groups):
                rows = min(128, shard - g * 128)
                h_t = hpool.tile([128, DP], F32, tag="h")
                nc.sync.dma_start(h_t[:rows, :], h0_d[g * 128:g * 128 + rows, :])
                transform(h_t, g, 0, 0)
            allgathers(0)

            # ---- GCN layers ----
            for l in range(n_layers):
                nxt = (l + 1) % 2
                tabs = (tab_a[l], tab_b[l])
                for g in range(groups):
                    rows = min(128, shard - g * 128)
                    n_mm = int(T[g, 0] + T[g, 1])
                    ps_a = psagg.tile([128, D], F32, tag="agg")
                    mm_i = 0
                    for tbl in (0, 1):
                        tcnt = int(T[g, tbl])
                        if tcnt == 0:
                            continue
                        soff = int(off[g, tbl])
                        g_t = gpool.tile([128, tcnt, DP], BF, tag="gbuf")
                        nc.gpsimd.dma_gather(
                            out_ap=g_t[:],
                            in_ap=tabs[tbl][:],
                            idxs_ap=idx_t[:, soff * 8:(soff + tcnt) * 8],
                            num_idxs=tcnt * 128,
                            num_idxs_reg=tcnt * 128,
                            elem_size=DP,
                        )
                        for t in range(tcnt):
                            s = soff + t
                            s_t = spool.tile([128, 128], BF, tag="stile")
                            nc.vector.tensor_scalar(
                                s_t[:], iota_f[:], dst_t[:, s:s + 1],
                                nrm_t[:, s:s + 1],
                                mybir.AluOpType.is_equal, mybir.AluOpType.mult)
                            nc.tensor.matmul(ps_a[:], s_t[:], g_t[:, t, :D],
                                             start=(mm_i == 0),
                                             stop=(mm_i == n_mm - 1))
                            mm_i += 1
                    # LayerNorm + GELU (conv bias and LN affine are identity
                    # in this model's params -- verified host-side)
                    msum = smallp.tile([128, 1], F32, tag="msum")
                    nc.vector.reduce_sum(msum[:], ps_a[:], axis=mybir.AxisListType.X)
                    mean = smallp.tile([128, 1], F32, tag="mean")
                    nc.scalar.mul(mean[:], msum[:], 1.0 / D)
                    scr = smallp.tile([128, D], F32, tag="lnscr")
                    sqs = smallp.tile([128, 1], F32, tag="sqs")
                    nc.scalar.activation(scr[:], ps_a[:],
                                         mybir.ActivationFunctionType.Square,
                                         accum_out=sqs[:])
                    var = smallp.tile([128, 1], F32, tag="var")
                    # var = sqs/D - mean^2
                    m2 = smallp.tile([128, 1], F32, tag="m2")
                    nc.vector.tensor_tensor(m2[:], mean[:], mean[:],
                                            mybir.AluOpType.mult)
                    nc.vector.tensor_scalar(var[:], sqs[:], 1.0 / D, m2[:],
                                            mybir.AluOpType.mult,
                                            mybir.AluOpType.subtract)
                    std = smallp.tile([128, 1], F32, tag="std")
                    nc.scalar.activation(std[:], var[:],
                                         mybir.ActivationFunctionType.Sqrt,
                                         bias=eps_t[:])
                    rstd = smallp.tile([128, 1], F32, tag="rstd")
                    nc.vector.reciprocal(rstd[:], std[:])
                    nbias = smallp.tile([128, 1], F32, tag="nbias")
                    nc.vector.tensor_scalar(nbias[:], mean[:], rstd[:], -1.0,
                                            mybir.AluOpType.mult,
                                            mybir.AluOpType.mult)
                    h_t = hpool.tile([128, DP], F32, tag="h")
                    nc.scalar.activation(h_t[:, :D], ps_a[:],
                                         mybir.ActivationFunctionType.Gelu,
                                         bias=nbias[:], scale=rstd[:])
                    if l < n_layers - 1:
                        nc.vector.memset(h_t[:, D:], 0.0)
                        transform(h_t, g, l + 1, nxt)
                    else:
                        nc.tensor.matmul(ps_pool[:], ones_t[:], h_t[:, :D],
                                         start=(g == 0), stop=(g == groups - 1))
                if l < n_layers - 1:
                    allgathers(l + 1)

            pool_s = constp.tile([1, D], F32)
            nc.vector.tensor_copy(pool_s[:], ps_pool[:])
            nc.sync.dma_start(pool_o[:], pool_s[:])
            if debug_tab:
                nc.sync.dma_start(dbg_o[:rows_a, :], tab_a[n_layers - 1][:])
                nc.sync.dma_start(dbg_o[rows_a:, :], tab_b[n_layers - 1][:])

    nc.compile()
    return nc


# ----------------------------------------------------------------------------
# driver
# ----------------------------------------------------------------------------

_CACHE = {}


def _get_program(key, *args, **kwargs):
    if key not in _CACHE:
        _CACHE[key] = _build_program(*args, **kwargs)
    return _CACHE[key]


def _run(h0, edge_index, wmats, n_nodes, ncore, n_layers, trace=False,
         debug_tab=False):
    """h0: [n_nodes, D] f32; wmats: [n_layers, D, D] (conv weight W, row-major
    as in params: hw = h @ W.T). Returns (pool_sum [D], results|None, bkr)."""
    shard = n_nodes // ncore
    src, dst, norm = _gcn_edges(edge_index, n_nodes)
    T, off, nslots, idx_w, dst_c, nrm_c = _build_meta(src, dst, norm,
                                                      n_nodes, ncore)
    key = (n_nodes, ncore, n_layers, nslots, T.tobytes(), debug_tab)
    nc = _get_program(key, n_nodes, ncore, n_layers, T, off, nslots,
                      debug_tab=debug_tab)

    # weights: wts[l, b, i, o] = W_l[o, b*128+i] (i.e. W^T blocks, zero-padded)
    wts = np.zeros((128, n_layers, 3, D), np.float32)
    for l in range(n_layers):
        wT = np.asarray(wmats[l], np.float32).T        # [in, out]
        for b in range(3):
            blk = wT[b * 128:min((b + 1) * 128, D), :]
            wts[:blk.shape[0], l, b, :] = blk

    h0p = np.zeros((n_nodes, DP), np.float32)
    h0p[:, :D] = h0
    ident = np.eye(128, dtype=np.float32)
    ones = np.ones((128, 1), np.float32)

    in_maps = []
    for c in range(ncore):
        in_maps.append({
            "h0s": np.ascontiguousarray(h0p[c * shard:(c + 1) * shard]),
            "idx16": idx_w[c],
            "dstloc": dst_c[c],
            "nrm": nrm_c[c],
            "wts": wts,
            "ident": ident,
            "onescol": ones,
        })
    bkr = run_bass_kernel_spmd(nc, in_maps, list(range(ncore)), trace=trace)
    pool_sum = np.zeros(D, np.float64)
    for c in range(ncore):
        pool_sum += bkr.results[c]["pool_out"][0].astype(np.float64)
    return pool_sum.astype(np.float32), bkr


def kernel(state, scores, edge_index, items_ready_to_cache, params):
    state = np.asarray(state, np.float32)
    scores = np.asarray(scores, np.float32)
    edge_index = np.asarray(edge_index)
    attn1, h0 = _prologue(state, scores, params)

    # device handles only the identity-affine LN / zero-bias conv case;
    # this model's params satisfy it (fall back would need extra plumbing)
    for cp in params["convs"]:
        assert not np.any(np.asarray(cp["b"])), "nonzero conv bias unsupported"
        assert np.allclose(np.asarray(cp["ln"]["g"]), 1.0)
        assert not np.any(np.asarray(cp["ln"]["b"]))

    wmats = np.stack([np.asarray(cp["w"], np.float32)
                      for cp in params["convs"]])
    pool_sum, _ = _run(h0, edge_index, wmats, N_NODES, NCORE, NLAYERS)
    return _epilogue(pool_sum, attn1, params, N_NODES)
